# revision 12
# baseline (speedup 1.0000x reference)
"""GATv2 x2 + edge decoder (gnn_message_passing) on 8 TRN2 NeuronCores.

Strategy (dst-sharded edge phase):
- Edges (incl. self-loops) are sorted by dst on the host and partitioned into
  per-dst-tile (128 nodes) contiguous runs, padded to a uniform number of
  128-edge subtiles (S_sub) so the SPMD program is identical on every core.
- Core k owns dst nodes [k*N/8, (k+1)*N/8): segment softmax/sum stay local.
- Per layer: xl = x@Wl.T+bl is computed (replicated GEMM) into a DRAM table;
  per edge-subtile xl[src] rows are fetched with indirect DMA (4KB/row).
  xr is never materialized: within a dst tile, xr[dst_e] is expanded from the
  128-node xr tile with a PE matmul against a selection matrix
  sel[e,n] = (dstloc_e == n), which also performs the segment-sum scatter
  (out += sel.T @ msg) and denominator (den += sel.T @ ex) as matmuls.
- Segment softmax skips the segment-max subtraction (mathematically identical;
  scores are O(1) here so exp cannot overflow).
- Between layers, z.T slices are AllGathered so every core can run the
  replicated GEMMs of the next layer.
- Decoder: zc@Wd1.T splits into P[src]+Q[dst] with P = z@Wd1[:, :C].T + bd1,
  Q = z@Wd1[:, C:].T (per-node GEMMs), then per-edge gather/add/lrelu/dot.
"""

import sys

sys.path.insert(0, "/opt/trn_rl_repo")

import numpy as np

import bass_rust
import concourse.bass as bass
import concourse.mybir as mybir
import concourse.tile as tile

P = 128
NS_ATT = 0.2
NS_ACT = 0.01
dt = mybir.dt
Alu = mybir.AluOpType
Act = mybir.ActivationFunctionType


# ---------------------------------------------------------------------------
# workaround: this walrus build rejects sem waits attached to InstDrain
# ("Too many sync wait commands"); hoist every drain wait onto NoOps.
def _fix_waits(nc, max_other=1):
    for bb in nc.main_func.blocks:
        newlist = []
        for ins in bb.instructions:
            si = ins.sync_info
            if si is not None and si.on_wait:
                waits = list(si.on_wait)
                no_wait = isinstance(ins, mybir.InstDrain) or hasattr(ins, "isa_opcode")
                limit = 0 if no_wait else max_other
                if len(waits) > limit:
                    nkeep = limit
                    extra = waits[: len(waits) - nkeep] if nkeep else waits
                    keep = waits[len(waits) - nkeep:] if nkeep else []
                    k = 0
                    while extra:
                        chunk, extra = extra[:1], extra[1:]
                        nop = mybir.InstNoOp(
                            name=f"{ins.name}_ws{k}", engine=ins.engine, ins=[], outs=[]
                        )
                        nop.sync_info = bass_rust.SyncInfo(on_wait=chunk, on_update=[])
                        newlist.append(nop)
                        k += 1
                    ins.sync_info = bass_rust.SyncInfo(
                        on_wait=keep, on_update=list(si.on_update or [])
                    )
            newlist.append(ins)
        bb.instructions = newlist


def _chunks(total, step=512):
    return [(i, min(i + step, total)) for i in range(0, total, step)]


def build_program(cfg, fix=True, dbg=False):
    """Build the SPMD Bass program. cfg keys:
    N, C (=IN=out_c), H, n_cores, S_sub, DSUB (decoder subtiles/core),
    GB (edge gather batch), GBD (decoder gather batch), bd2 (float).
    """
    N, C, H = cfg["N"], cfg["C"], cfg["H"]
    NC_ = cfg["n_cores"]
    HC = H * C
    NPC = N // NC_        # nodes per core
    T = NPC // P          # dst tiles per core
    TN = N // P           # total node tiles
    S = cfg["S_sub"]
    DSUB = cfg["DSUB"]
    GB = cfg["GB"]
    GBD = cfg["GBD"]

    nc = bass.Bass()

    def inp(name, shape, d=dt.float32):
        return nc.declare_dram_parameter(name, list(shape), d, isOutput=False)

    x_T = inp("x_T", [C, N])
    x_own_T = inp("x_own_T", [C, NPC])
    srcT = inp("srcT", [P, T * S], dt.int32)
    dstlocT = inp("dstlocT", [P, T * S])
    dec_srcT = inp("dec_srcT", [P, DSUB], dt.int32)
    dec_dstT = inp("dec_dstT", [P, DSUB], dt.int32)
    wl1T = inp("wl1T", [C, HC])
    wr1T = inp("wr1T", [C, HC])
    wl2T = inp("wl2T", [C, HC])
    wr2T = inp("wr2T", [C, HC])
    blr1 = inp("blr1", [1, HC])
    brr1 = inp("brr1", [1, HC])
    blr2 = inp("blr2", [1, HC])
    brr2 = inp("brr2", [1, HC])
    att1_t = inp("att1_t", [P, HC])
    att2_t = inp("att2_t", [P, HC])
    bias1_t = inp("bias1_t", [P, C])
    bias2_t = inp("bias2_t", [P, C])
    a_t = inp("a_t", [C, C])
    b_t = inp("b_t", [C, C])
    bd1r = inp("bd1r", [1, C])
    wd2_t = inp("wd2_t", [P, C])
    iota_t = inp("iota_t", [P, P])
    ident_t = inp("ident_t", [P, P])
    ones_t = inp("ones_t", [1, P])

    out_dec = nc.declare_dram_parameter("out_dec", [DSUB, P], dt.float32, isOutput=True)
    if dbg:
        dbg_xl1 = nc.declare_dram_parameter("dbg_xl1", [2 * P, HC], dt.float32, isOutput=True)
        dbg_zT = nc.declare_dram_parameter("dbg_zT", [C, NPC], dt.float32, isOutput=True)
        dbg_zall = nc.declare_dram_parameter("dbg_zall", [NC_ * C, NPC], dt.float32, isOutput=True)
        dbg_xl2 = nc.declare_dram_parameter("dbg_xl2", [2 * P, HC], dt.float32, isOutput=True)
        dbg_z2T = nc.declare_dram_parameter("dbg_z2T", [C, NPC], dt.float32, isOutput=True)
        dbg_p = nc.declare_dram_parameter("dbg_p", [2 * P, C], dt.float32, isOutput=True)
        dbg_q = nc.declare_dram_parameter("dbg_q", [2 * P, C], dt.float32, isOutput=True)
        dbg_xr = nc.declare_dram_parameter("dbg_xr", [P, HC], dt.float32, isOutput=True)
        dbg_sel = nc.declare_dram_parameter("dbg_sel", [P, P], dt.float32, isOutput=True)
        dbg_selT = nc.declare_dram_parameter("dbg_selT", [P, P], dt.float32, isOutput=True)
        dbg_lr = nc.declare_dram_parameter("dbg_lr", [P, HC], dt.float32, isOutput=True)
        dbg_s8 = nc.declare_dram_parameter("dbg_s8", [P, H], dt.float32, isOutput=True)
        dbg_ex = nc.declare_dram_parameter("dbg_ex", [P, H], dt.float32, isOutput=True)
        dbg_msg = nc.declare_dram_parameter("dbg_msg", [P, HC], dt.float32, isOutput=True)
        dbg_den = nc.declare_dram_parameter("dbg_den", [P, H], dt.float32, isOutput=True)
        dbg_osb = nc.declare_dram_parameter("dbg_osb", [P, HC], dt.float32, isOutput=True)

    with tile.TileContext(nc) as tc:
        with (
            tc.tile_pool(name="const", bufs=1) as cp,
            tc.tile_pool(name="work", bufs=2) as wp,
            tc.tile_pool(name="gpool", bufs=3) as gp,
            tc.tile_pool(name="psum", bufs=2, space="PSUM") as pp,
            tc.tile_pool(name="psum1", bufs=1, space="PSUM") as pp1,
            tc.tile_pool(name="dram", bufs=1, space="DRAM") as dp,
        ):
            # ---- load constants -------------------------------------------------
            def cload(ap, shape, d=dt.float32):
                t_ = cp.tile(list(shape), d, tag=f"c_{ap.name}")
                nc.sync.dma_start(out=t_[:], in_=ap[:, :])
                return t_

            wl1_sb = cload(wl1T, [C, HC])
            wr1_sb = cload(wr1T, [C, HC])
            wl2_sb = cload(wl2T, [C, HC])
            wr2_sb = cload(wr2T, [C, HC])
            blr1_sb = cload(blr1, [1, HC])
            brr1_sb = cload(brr1, [1, HC])
            blr2_sb = cload(blr2, [1, HC])
            brr2_sb = cload(brr2, [1, HC])
            att1_sb = cload(att1_t, [P, HC])
            att2_sb = cload(att2_t, [P, HC])
            bias1_sb = cload(bias1_t, [P, C])
            bias2_sb = cload(bias2_t, [P, C])
            a_sb = cload(a_t, [C, C])
            b_sb = cload(b_t, [C, C])
            bd1_sb = cload(bd1r, [1, C])
            wd2_sb = cload(wd2_t, [P, C])
            iota_sb = cload(iota_t, [P, P])
            ident_sb = cload(ident_t, [P, P])
            ones_sb = cload(ones_t, [1, P])
            srcT_sb = cload(srcT, [P, T * S], dt.int32)
            dstloc_sb = cload(dstlocT, [P, T * S])
            dsrc_sb = cload(dec_srcT, [P, DSUB], dt.int32)
            ddst_sb = cload(dec_dstT, [P, DSUB], dt.int32)
            xown_sb = cload(x_own_T, [C, NPC])
            res_sb = cp.tile([P, DSUB], dt.float32, tag="res_sb")

            # ---- DRAM scratch ---------------------------------------------------
            xl1_tab = dp.tile([N, HC], dt.float32)
            xl2_tab = dp.tile([N, HC], dt.float32)
            zT_loc = dp.tile([C, NPC], dt.float32)
            zT_all = dp.tile([NC_, C, NPC], dt.float32)
            z2T_loc = dp.tile([C, NPC], dt.float32)
            z2T_all = dp.tile([NC_, C, NPC], dt.float32)
            p_tab = dp.tile([N, C], dt.float32)
            q_tab = dp.tile([N, C], dt.float32)

            def biased_gemm(lhsT_sb, wT_sb, brow_sb, ps, width):
                """ps[P, width] = bias_row + lhsT.T @ wT  (chunked by 512)."""
                for n0, n1 in _chunks(width):
                    nc.tensor.matmul(out=ps[:, n0:n1], lhsT=ones_sb[:, :],
                                     rhs=brow_sb[:, n0:n1], start=True, stop=False)
                    nc.tensor.matmul(out=ps[:, n0:n1], lhsT=lhsT_sb[:],
                                     rhs=wT_sb[:, n0:n1], start=False, stop=True)

            # ---- phase A/D: xl table GEMM (replicated over nodes) ---------------
            def dense_phase(wT_sb, brow_sb, xl_tab, src_of_tile):
                for nt in range(TN):
                    lt = wp.tile([C, P], dt.float32, tag="lhsT")
                    nc.sync.dma_start(out=lt[:], in_=src_of_tile(nt))
                    ps = pp.tile([P, HC], dt.float32, tag="big")
                    biased_gemm(lt, wT_sb, brow_sb, ps, HC)
                    xsb = gp.tile([P, HC], dt.float32, tag="xl_out")
                    nc.scalar.activation(out=xsb[:], in_=ps[:], func=Act.Copy)
                    nc.sync.dma_start(out=xl_tab[nt * P:(nt + 1) * P, :], in_=xsb[:])

            # ---- phase B/E: edge phase (dst-sharded) ----------------------------
            def edge_phase(xl_tab, wrT_sb, brow_sb, att_sb, bias_sb, zT_out, act):
                tap = dbg and xl_tab is xl1_tab
                inv_h = 1.0 / H
                for t in range(T):
                    tap_t = tap and t == 0
                    xo = wp.tile([C, P], dt.float32, tag="lhsT")
                    if xl_tab is xl1_tab:
                        nc.sync.dma_start(out=xo[:], in_=xown_sb[:, t * P:(t + 1) * P])
                    else:
                        nc.sync.dma_start(out=xo[:], in_=zT_loc[:, t * P:(t + 1) * P])
                    psx = pp.tile([P, HC], dt.float32, tag="big")
                    biased_gemm(xo, wrT_sb, brow_sb, psx, HC)
                    xr_sb = wp.tile([P, HC], dt.float32, tag="xr")
                    nc.scalar.activation(out=xr_sb[:], in_=psx[:], func=Act.Copy)
                    if tap_t:
                        nc.sync.dma_start(out=dbg_xr[:, :], in_=xr_sb[:])

                    out_ps = pp1.tile([P, HC], dt.float32, tag="out")
                    den_ps = pp1.tile([P, H], dt.float32, tag="den")

                    for c0 in range(0, S, GB):
                        gb = min(GB, S - c0)
                        gbuf = gp.tile([P, GB * HC], dt.float32, tag="gbuf")
                        col0 = t * S + c0
                        nc.gpsimd.indirect_dma_start(
                            out=gbuf[:, : gb * HC], out_offset=None,
                            in_=xl_tab[:, :],
                            in_offset=bass.IndirectOffsetOnAxis(
                                ap=srcT_sb[:, col0:col0 + gb], axis=0),
                        )
                        for g in range(gb):
                            s = c0 + g
                            col = t * S + s
                            xl_g = gbuf[:, g * HC:(g + 1) * HC]
                            sel = wp.tile([P, P], dt.float32, tag="sel")
                            nc.vector.tensor_tensor(
                                out=sel[:],
                                in0=dstloc_sb[:, col:col + 1].to_broadcast([P, P]),
                                in1=iota_sb[:], op=Alu.is_equal)
                            selT_ps = pp1.tile([P, P], dt.float32, tag="small")
                            nc.tensor.transpose(out=selT_ps[:], in_=sel[:],
                                                identity=ident_sb[:])
                            selT = wp.tile([P, P], dt.float32, tag="selT")
                            nc.scalar.activation(out=selT[:], in_=selT_ps[:],
                                                 func=Act.Copy)
                            if tap_t and s == 0:
                                nc.sync.dma_start(out=dbg_sel[:, :], in_=sel[:])
                                nc.sync.dma_start(out=dbg_selT[:, :], in_=selT[:])
                            m_ps = pp.tile([P, HC], dt.float32, tag="big")
                            for n0, n1 in _chunks(HC):
                                nc.tensor.matmul(out=m_ps[:, n0:n1], lhsT=selT[:],
                                                 rhs=xr_sb[:, n0:n1],
                                                 start=True, stop=False)
                                nc.tensor.matmul(out=m_ps[:, n0:n1], lhsT=ident_sb[:],
                                                 rhs=xl_g[:, n0:n1],
                                                 start=False, stop=True)
                            # lrelu(m) = max(m, 0.2*m); the ISA allows only one
                            # PSUM operand per DVE op, so scale on ACT first.
                            msc = wp.tile([P, HC], dt.float32, tag="msc")
                            nc.scalar.activation(out=msc[:], in_=m_ps[:],
                                                 func=Act.Copy, scale=NS_ATT)
                            q = wp.tile([P, HC], dt.float32, tag="q")
                            nc.vector.tensor_tensor(out=q[:], in0=m_ps[:],
                                                    in1=msc[:], op=Alu.max)
                            tq = wp.tile([P, HC], dt.float32, tag="tq")
                            nc.vector.tensor_tensor(out=tq[:], in0=q[:], in1=att_sb[:],
                                                    op=Alu.mult)
                            s8 = wp.tile([P, H], dt.float32, tag="s8")
                            nc.vector.tensor_reduce(
                                out=s8[:], in_=tq[:].rearrange("p (h c) -> p h c", h=H),
                                axis=mybir.AxisListType.X, op=Alu.add)
                            ex = wp.tile([P, H], dt.float32, tag="ex")
                            nc.scalar.activation(out=ex[:], in_=s8[:], func=Act.Exp)
                            if tap_t and s == 0:
                                nc.sync.dma_start(out=dbg_lr[:, :], in_=q[:])
                                nc.sync.dma_start(out=dbg_s8[:, :], in_=s8[:])
                                nc.sync.dma_start(out=dbg_ex[:, :], in_=ex[:])
                            msg = wp.tile([P, HC], dt.float32, tag="msg")
                            nc.vector.tensor_tensor(
                                out=msg[:].rearrange("p (h c) -> p h c", h=H),
                                in0=xl_g.rearrange("p (h c) -> p h c", h=H),
                                in1=ex[:].to_broadcast([P, H, C]), op=Alu.mult)
                            if tap_t and s == 0:
                                nc.sync.dma_start(out=dbg_msg[:, :], in_=msg[:])
                            first, last = (s == 0), (s == S - 1)
                            for n0, n1 in _chunks(HC):
                                nc.tensor.matmul(out=out_ps[:, n0:n1], lhsT=sel[:],
                                                 rhs=msg[:, n0:n1],
                                                 start=first, stop=last)
                            nc.tensor.matmul(out=den_ps[:], lhsT=sel[:], rhs=ex[:],
                                             start=first, stop=last)

                    if tap_t:
                        den_sb = wp.tile([P, H], dt.float32, tag="densb")
                        nc.vector.tensor_copy(out=den_sb[:], in_=den_ps[:])
                        nc.sync.dma_start(out=dbg_den[:, :], in_=den_sb[:])
                    rden = wp.tile([P, H], dt.float32, tag="rden")
                    nc.vector.reciprocal(out=rden[:], in_=den_ps[:])
                    o_sb = wp.tile([P, HC], dt.float32, tag="o")
                    nc.vector.tensor_tensor(
                        out=o_sb[:].rearrange("p (h c) -> p h c", h=H),
                        in0=out_ps[:].rearrange("p (h c) -> p h c", h=H),
                        in1=rden[:].to_broadcast([P, H, C]), op=Alu.mult)
                    if tap_t:
                        nc.sync.dma_start(out=dbg_osb[:, :], in_=o_sb[:])
                    zsum = wp.tile([P, C], dt.float32, tag="zsum")
                    nc.vector.tensor_reduce(
                        out=zsum[:], in_=o_sb[:].rearrange("p (h c) -> p c h", h=H),
                        axis=mybir.AxisListType.X, op=Alu.add)
                    zt = wp.tile([P, C], dt.float32, tag="zt")
                    nc.vector.scalar_tensor_tensor(
                        out=zt[:], in0=zsum[:], scalar=inv_h, in1=bias_sb[:],
                        op0=Alu.mult, op1=Alu.add)
                    if act:
                        zt2 = wp.tile([P, C], dt.float32, tag="zt2")
                        nc.vector.scalar_tensor_tensor(
                            out=zt2[:], in0=zt[:], scalar=NS_ACT, in1=zt[:],
                            op0=Alu.mult, op1=Alu.max)
                    else:
                        zt2 = zt
                    ztp = pp1.tile([P, C], dt.float32, tag="small")
                    nc.tensor.transpose(out=ztp[:, :], in_=zt2[:], identity=ident_sb[:])
                    ztsb = wp.tile([C, P], dt.float32, tag="ztsb")
                    nc.scalar.activation(out=ztsb[:], in_=ztp[:, :], func=Act.Copy)
                    nc.sync.dma_start(out=zT_out[:, t * P:(t + 1) * P], in_=ztsb[:])

            # ------------------ pipeline ------------------
            PH = cfg.get("_phases", "ABCDEFGH")
            if "A" in PH:
                dense_phase(wl1_sb, blr1_sb, xl1_tab,
                            lambda nt: x_T[:, nt * P:(nt + 1) * P])
            if "B" in PH:
                edge_phase(xl1_tab, wr1_sb, brr1_sb, att1_sb, bias1_sb, zT_loc, act=True)
            if "C" in PH:
                nc.gpsimd.collective_compute(
                    "AllGather", Alu.bypass,
                    replica_groups=[list(range(NC_))],
                    ins=[zT_loc.opt()], outs=[zT_all.opt()])
            if "D" in PH:
                dense_phase(wl2_sb, blr2_sb, xl2_tab,
                            lambda nt: zT_all[nt // T, :, (nt % T) * P:(nt % T + 1) * P])
            if "E" in PH:
                edge_phase(xl2_tab, wr2_sb, brr2_sb, att2_sb, bias2_sb, z2T_loc, act=False)
            if "F" in PH:
                nc.gpsimd.collective_compute(
                    "AllGather", Alu.bypass,
                    replica_groups=[list(range(NC_))],
                    ins=[z2T_loc.opt()], outs=[z2T_all.opt()])

            if dbg:
                nc.gpsimd.dma_start(out=dbg_xl1[:, :], in_=xl1_tab[0:2 * P, :])
                nc.gpsimd.dma_start(out=dbg_zT[:, :], in_=zT_loc[:, :])
                nc.gpsimd.dma_start(
                    out=dbg_zall[:, :],
                    in_=zT_all[:, :, :].rearrange("k c n -> (k c) n"))
                nc.gpsimd.dma_start(out=dbg_xl2[:, :], in_=xl2_tab[0:2 * P, :])
                nc.gpsimd.dma_start(out=dbg_z2T[:, :], in_=z2T_loc[:, :])

            # ---- phase G: P/Q tables -------------------------------------------
            for nt in (range(TN) if "G" in PH else []):
                lt = wp.tile([C, P], dt.float32, tag="lhsT")
                nc.sync.dma_start(
                    out=lt[:],
                    in_=z2T_all[nt // T, :, (nt % T) * P:(nt % T + 1) * P])
                psp = pp1.tile([P, C], dt.float32, tag="small")
                nc.tensor.matmul(out=psp[:], lhsT=ones_sb[:, :], rhs=bd1_sb[:, :],
                                 start=True, stop=False)
                nc.tensor.matmul(out=psp[:], lhsT=lt[:], rhs=a_sb[:],
                                 start=False, stop=True)
                p_sb = wp.tile([P, C], dt.float32, tag="pq_out")
                nc.scalar.activation(out=p_sb[:], in_=psp[:], func=Act.Copy)
                nc.sync.dma_start(out=p_tab[nt * P:(nt + 1) * P, :], in_=p_sb[:])
                psq = pp1.tile([P, C], dt.float32, tag="small")
                nc.tensor.matmul(out=psq[:], lhsT=lt[:], rhs=b_sb[:],
                                 start=True, stop=True)
                q_sb = wp.tile([P, C], dt.float32, tag="pq_out")
                nc.scalar.activation(out=q_sb[:], in_=psq[:], func=Act.Copy)
                nc.sync.dma_start(out=q_tab[nt * P:(nt + 1) * P, :], in_=q_sb[:])

            if dbg:
                nc.gpsimd.dma_start(out=dbg_p[:, :], in_=p_tab[0:2 * P, :])
                nc.gpsimd.dma_start(out=dbg_q[:, :], in_=q_tab[0:2 * P, :])

            # ---- phase H: decoder ----------------------------------------------
            bd2 = float(cfg["bd2"])
            for j0 in (range(0, DSUB, GBD) if "H" in PH else []):
                gbd = min(GBD, DSUB - j0)
                pg = gp.tile([P, GBD * C], dt.float32, tag="pg")
                nc.gpsimd.indirect_dma_start(
                    out=pg[:, : gbd * C], out_offset=None, in_=p_tab[:, :],
                    in_offset=bass.IndirectOffsetOnAxis(
                        ap=dsrc_sb[:, j0:j0 + gbd], axis=0))
                qg = gp.tile([P, GBD * C], dt.float32, tag="qg")
                nc.gpsimd.indirect_dma_start(
                    out=qg[:, : gbd * C], out_offset=None, in_=q_tab[:, :],
                    in_offset=bass.IndirectOffsetOnAxis(
                        ap=ddst_sb[:, j0:j0 + gbd], axis=0))
                for g in range(gbd):
                    j = j0 + g
                    u = wp.tile([P, C], dt.float32, tag="u")
                    nc.vector.tensor_tensor(out=u[:], in0=pg[:, g * C:(g + 1) * C],
                                            in1=qg[:, g * C:(g + 1) * C], op=Alu.add)
                    d = wp.tile([P, C], dt.float32, tag="d")
                    nc.vector.scalar_tensor_tensor(
                        out=d[:], in0=u[:], scalar=NS_ACT, in1=u[:],
                        op0=Alu.mult, op1=Alu.max)
                    dw = wp.tile([P, C], dt.float32, tag="dw")
                    nc.vector.tensor_tensor(out=dw[:], in0=d[:], in1=wd2_sb[:],
                                            op=Alu.mult)
                    nc.vector.tensor_reduce(out=res_sb[:, j:j + 1], in_=dw[:],
                                            axis=mybir.AxisListType.X, op=Alu.add)

            # write result: transpose res_sb [P, DSUB] into out_dec [DSUB, P]
            for b0 in (range(0, DSUB, P) if "H" in PH else []):
                bw = min(P, DSUB - b0)
                rp = pp1.tile([P, P], dt.float32, tag="small")
                nc.tensor.transpose(out=rp[:bw, :], in_=res_sb[:, b0:b0 + bw],
                                    identity=ident_sb[:])
                rsb = wp.tile([P, P], dt.float32, tag="res_out")
                nc.scalar.activation(out=rsb[:bw, :], in_=rp[:bw, :], func=Act.Copy)
                nc.sync.dma_start(out=out_dec[b0:b0 + bw, :], in_=rsb[:bw, :])

    if fix:
        _fix_waits(nc)
    return nc


def build_truncated(cfg, phases, fix=True):
    """Build with only a prefix of phases, for timing bisection.
    phases: string subset-prefix of "ABCDEFGH"."""
    cfg = dict(cfg)
    cfg["_phases"] = phases
    return build_program(cfg, fix=fix)


# ---------------------------------------------------------------------------
def host_prep(inputs, n_cores=8, GB=1, GBD=1):
    """Host-side preprocessing: edge sort/pad, weight transposes, per-core maps."""
    x = np.ascontiguousarray(np.asarray(inputs["x"], dtype=np.float32))
    N, C = x.shape
    Wl1 = np.asarray(inputs["Wl1"], np.float32)
    H = Wl1.shape[0] // C
    HC = H * C
    NPC = N // n_cores
    T = NPC // P

    ei = np.asarray(inputs["edge_index"])
    src = ei[0].astype(np.int64)
    dst = ei[1].astype(np.int64)
    E = src.shape[0]
    loops = np.arange(N, dtype=np.int64)
    src_a = np.concatenate([src, loops])
    dst_a = np.concatenate([dst, loops])
    order = np.argsort(dst_a, kind="stable")
    src_s, dst_s = src_a[order], dst_a[order]

    TN = N // P
    tile_id = dst_s // P
    counts = np.bincount(tile_id, minlength=TN)
    S_sub = int(np.ceil(counts.max() / P))
    cap = S_sub * P
    src_pad = np.zeros((TN, cap), np.int32)
    dstloc_pad = np.full((TN, cap), -1.0, np.float32)
    off = np.concatenate([[0], np.cumsum(counts)])
    for t in range(TN):
        c = counts[t]
        src_pad[t, :c] = src_s[off[t]:off[t] + c]
        dstloc_pad[t, :c] = (dst_s[off[t]:off[t] + c] - t * P).astype(np.float32)

    E_dec = E // n_cores
    assert E % n_cores == 0 and E_dec % P == 0
    DSUB = E_dec // P

    def tr(a):  # -> f32 transposed contiguous
        return np.ascontiguousarray(np.asarray(a, np.float32).T)

    Wd1 = np.asarray(inputs["Wd1"], np.float32)
    shared = {
        "x_T": tr(x),
        "wl1T": tr(inputs["Wl1"]), "wr1T": tr(inputs["Wr1"]),
        "wl2T": tr(inputs["Wl2"]), "wr2T": tr(inputs["Wr2"]),
        "blr1": np.asarray(inputs["bl1"], np.float32).reshape(1, HC),
        "brr1": np.asarray(inputs["br1"], np.float32).reshape(1, HC),
        "blr2": np.asarray(inputs["bl2"], np.float32).reshape(1, HC),
        "brr2": np.asarray(inputs["br2"], np.float32).reshape(1, HC),
        "att1_t": np.tile(np.asarray(inputs["att1"], np.float32).reshape(1, HC), (P, 1)),
        "att2_t": np.tile(np.asarray(inputs["att2"], np.float32).reshape(1, HC), (P, 1)),
        "bias1_t": np.tile(np.asarray(inputs["bias1"], np.float32).reshape(1, C), (P, 1)),
        "bias2_t": np.tile(np.asarray(inputs["bias2"], np.float32).reshape(1, C), (P, 1)),
        "a_t": np.ascontiguousarray(Wd1[:, :C].T),
        "b_t": np.ascontiguousarray(Wd1[:, C:].T),
        "bd1r": np.asarray(inputs["bd1"], np.float32).reshape(1, C),
        "wd2_t": np.tile(np.asarray(inputs["Wd2"], np.float32).reshape(1, C), (P, 1)),
        "iota_t": np.tile(np.arange(P, dtype=np.float32)[None, :], (P, 1)),
        "ident_t": np.eye(P, dtype=np.float32),
        "ones_t": np.ones((1, P), np.float32),
    }

    xt_full = shared["x_T"]
    in_maps = []
    for k in range(n_cores):
        tiles = slice(k * T, (k + 1) * T)
        src_k = np.ascontiguousarray(
            src_pad[tiles].reshape(T * S_sub, P).T)           # [P, T*S]
        dl_k = np.ascontiguousarray(
            dstloc_pad[tiles].reshape(T * S_sub, P).T)
        es = slice(k * E_dec, (k + 1) * E_dec)
        dsrc_k = np.ascontiguousarray(
            src[es].astype(np.int32).reshape(DSUB, P).T)
        ddst_k = np.ascontiguousarray(
            dst[es].astype(np.int32).reshape(DSUB, P).T)
        m = dict(shared)
        m["x_own_T"] = np.ascontiguousarray(xt_full[:, k * NPC:(k + 1) * NPC])
        m["srcT"] = src_k
        m["dstlocT"] = dl_k
        m["dec_srcT"] = dsrc_k
        m["dec_dstT"] = ddst_k
        in_maps.append(m)

    cfg = {
        "N": N, "C": C, "H": H, "n_cores": n_cores, "S_sub": S_sub,
        "DSUB": DSUB, "GB": GB, "GBD": GBD,
        "bd2": float(np.asarray(inputs["bd2"]).reshape(-1)[0]),
    }
    return in_maps, cfg


def kernel(**inputs):
    from concourse.bass_utils import run_bass_kernel_spmd

    n_cores = 8
    in_maps, cfg = host_prep(inputs, n_cores=n_cores)
    nc = build_program(cfg)
    res = run_bass_kernel_spmd(nc, in_maps, list(range(n_cores)))
    out = np.concatenate(
        [res.results[k]["out_dec"].reshape(-1) for k in range(n_cores)])
    return (out + cfg["bd2"]).astype(np.float32)
